# revision 1
# baseline (speedup 1.0000x reference)
"""Euclidean distance (cdist) kernel for Trainium2, 8 NeuronCores.

out[b, j] = || x[b, :] - weight[:, j] ||_2   for x [4096, 64], weight [64, 50000].

Sharding (per hint): K = 50000 split into 8 slabs of 6250, one per core
(tensor-parallel over prototypes); x replicated; no cross-core reduction.

Math: dist^2 = x2[b] + w2[j] - 2*x@w. The matmul runs in fp32r (the PE's
fast fp32 mode, RNE-rounded to 11 mantissa bits) at 4x the fp32 rate, with
full fp32-level accuracy recovered via a Dekker-style hi/lo split that
exploits the unused contraction capacity (D=64 of 128 partitions):

  mm1: lhsT=[xs_hi; xs_lo] (128 rows) rhs=[w_hi; w_hi]       -> -2x @ w_hi
  mm2: lhsT=[xs_hi; 1; 1]  (66 rows)  rhs=[w_lo; w2_hi; w2_lo]
                                              -> -2x @ w_lo + w2  (accum)
  where xs = -2x, v_hi = rne11(v), v_lo = rne11(v - v_hi).
  PSUM = -2*x'@w' + w2   with x', w' accurate to 22+ mantissa bits.
  ScalarE: out = sqrt(PSUM + x2[b])  (x2 as exact per-partition bias).

All hi/lo operands are rounded on the host (exact emulation of the HW's
fp32r RNE-11 rounding), shipped as float32r DRAM tensors.

Per core: 32 b-tiles of 128 rows; per b-tile 13 j-tiles of <=512 cols
(one PSUM bank); per b-tile a single contiguous 3.2 MB DMA store.
"""

import numpy as np
from contextlib import ExitStack

import concourse.bass as bass
import concourse.bacc as bacc
import concourse.tile as tile
from concourse import mybir
from concourse.bass_utils import run_bass_kernel_spmd

B, D, K = 4096, 64, 50000
NCORES = 8
KS = K // NCORES  # 6250 columns per core
P = 128
JT = 512          # matmul free-dim tile (one PSUM bank of fp32)
D2 = 2 * D        # 128: stacked hi/lo contraction for mm1
DL = D + 2        # 66: contraction for mm2 (w_lo + w2_hi + w2_lo rows)

F32 = mybir.dt.float32
F32R = mybir.dt.float32r


def build_nc(b=B, ks=KS):
    nbt = b // P
    nc = bacc.Bacc("TRN2", target_bir_lowering=False, debug=False)
    xs128 = nc.dram_tensor("xs128", [D2, b], F32R, kind="ExternalInput").ap()
    xs66 = nc.dram_tensor("xs66", [DL, b], F32R, kind="ExternalInput").ap()
    wst1 = nc.dram_tensor("wst1", [D2, ks], F32R, kind="ExternalInput").ap()
    wst2 = nc.dram_tensor("wst2", [DL, ks], F32R, kind="ExternalInput").ap()
    x2 = nc.dram_tensor("x2", [P, nbt], F32, kind="ExternalInput").ap()
    out = nc.dram_tensor("out", [b, ks], F32, kind="ExternalOutput").ap()

    CHUNK = 4 * JT  # 2048: one 4-bank PSUM tile, one ACT instruction
    chunks = [(c0, min(CHUNK, ks - c0)) for c0 in range(0, ks, CHUNK)]

    with tile.TileContext(nc) as tc:
        with ExitStack() as ctx:
            singles = ctx.enter_context(tc.tile_pool(name="singles", bufs=1))
            outp = ctx.enter_context(tc.tile_pool(name="outp", bufs=2))
            psum = ctx.enter_context(tc.tile_pool(name="psum", bufs=2, space="PSUM"))

            # Load order = criticality: the first j-tile's weights + x operands
            # gate the first matmuls; later weight chunks overlap with compute.
            wst1_sb = []
            wst2_sb = []
            for ic, (c0, cn) in enumerate(chunks):
                wst1_sb.append(singles.tile([D2, cn], F32R, name=f"wst1_{c0}"))
                wst2_sb.append(singles.tile([DL, cn], F32R, name=f"wst2_{c0}"))

            # chunk-0 weights and b-tile-0 x slices arrive first so the first
            # matmuls start as early as possible; the bulk follows.
            nc.sync.dma_start(out=wst1_sb[0][:, 0:JT], in_=wst1[:, 0:JT])
            xs128_sb = singles.tile([D2, b], F32R)
            nc.sync.dma_start(out=xs128_sb[:, 0:P], in_=xs128[:, 0:P])
            nc.sync.dma_start(out=wst2_sb[0][:, 0:JT], in_=wst2[:, 0:JT])
            xs66_sb = singles.tile([DL, b], F32R)
            nc.sync.dma_start(out=xs66_sb[:, 0:P], in_=xs66[:, 0:P])
            x2_sb = singles.tile([P, nbt], F32)
            nc.sync.dma_start(out=x2_sb, in_=x2)
            c0n = chunks[0][1]
            nc.sync.dma_start(out=wst1_sb[0][:, JT:c0n], in_=wst1[:, JT:c0n])
            nc.sync.dma_start(out=wst2_sb[0][:, JT:c0n], in_=wst2[:, JT:c0n])
            nc.sync.dma_start(out=xs128_sb[:, P:b], in_=xs128[:, P:b])
            nc.sync.dma_start(out=xs66_sb[:, P:b], in_=xs66[:, P:b])
            for ic, (c0, cn) in enumerate(chunks):
                if ic == 0:
                    continue
                nc.sync.dma_start(out=wst1_sb[ic], in_=wst1[:, c0:c0 + cn])
                nc.sync.dma_start(out=wst2_sb[ic], in_=wst2[:, c0:c0 + cn])

            for ib in range(nbt):
                # Store per chunk only on the first b-tile (starts the store
                # pipeline early); whole-row 3.2 MB stores otherwise — large
                # stores measurably minimize total DMA engine-seconds.
                chunked_store = ib == 0
                ot = outp.tile([P, ks], F32)
                for ic, (c0, cn) in enumerate(chunks):
                    pt = psum.tile([P, CHUNK], F32)
                    for jj in range(0, cn, JT):
                        jn = min(JT, cn - jj)
                        nc.tensor.matmul(
                            pt[:, jj:jj + jn],
                            xs128_sb[:, ib * P:(ib + 1) * P],
                            wst1_sb[ic][:, jj:jj + jn],
                            start=True,
                            stop=False,
                        )
                        nc.tensor.matmul(
                            pt[:, jj:jj + jn],
                            xs66_sb[:, ib * P:(ib + 1) * P],
                            wst2_sb[ic][:, jj:jj + jn],
                            start=False,
                            stop=True,
                        )
                    nc.scalar.activation(
                        ot[:, c0:c0 + cn],
                        pt[:, :cn],
                        mybir.ActivationFunctionType.Sqrt,
                        bias=x2_sb[:, ib:ib + 1],
                        scale=1.0,
                    )
                    if chunked_store:
                        nc.sync.dma_start(
                            out=out[ib * P:(ib + 1) * P, c0:c0 + cn],
                            in_=ot[:, c0:c0 + cn],
                        )
                if not chunked_store:
                    nc.sync.dma_start(out=out[ib * P:(ib + 1) * P, :], in_=ot)
    nc.compile()
    return nc


def _rne11(x):
    """HW-exact fp32r rounding: RNE to 11 mantissa bits."""
    x = np.asarray(x, np.float32)
    u = x.view(np.uint32).astype(np.uint64)
    shift = np.uint64(12)
    half = np.uint64(1 << 11)
    lsb = (u >> shift) & np.uint64(1)
    u2 = (u + half - np.uint64(1) + lsb) >> shift << shift
    return u2.astype(np.uint32).view(np.float32)


def prep_inputs(x, weight):
    """Host-side prep: hi/lo fp32r splits and stacked operand matrices."""
    x = np.ascontiguousarray(x, dtype=np.float32)
    weight = np.ascontiguousarray(weight, dtype=np.float32)
    b, d = x.shape
    k = weight.shape[1]
    x2 = (x.astype(np.float64) ** 2).sum(axis=1).astype(np.float32)
    w2 = (weight.astype(np.float64) ** 2).sum(axis=0).astype(np.float32)

    xs = (-2.0 * x).astype(np.float32)
    xs_hi = _rne11(xs)
    xs_lo = _rne11((xs - xs_hi).astype(np.float32))
    w_hi = _rne11(weight)
    w_lo = _rne11((weight - w_hi).astype(np.float32))
    w2_hi = _rne11(w2)
    w2_lo = _rne11((w2 - w2_hi).astype(np.float32))

    xs128 = np.empty((D2, b), dtype=np.float32)
    xs128[:d] = xs_hi.T
    xs128[d:] = xs_lo.T
    xs66 = np.empty((DL, b), dtype=np.float32)
    xs66[:d] = xs_hi.T
    xs66[d:] = 1.0
    wst1 = np.empty((D2, k), dtype=np.float32)
    wst1[:d] = w_hi
    wst1[d:] = w_hi
    wst2 = np.empty((DL, k), dtype=np.float32)
    wst2[:d] = w_lo
    wst2[d] = w2_hi
    wst2[d + 1] = w2_lo
    x2t = np.ascontiguousarray(x2.reshape(b // P, P).T)  # [P, NBT]
    return xs128, xs66, wst1, wst2, x2t


_nc_cache = {}


def _get_nc():
    if "nc" not in _nc_cache:
        _nc_cache["nc"] = build_nc()
    return _nc_cache["nc"]


def make_in_maps(x, weight, ks=KS):
    xs128, xs66, wst1, wst2, x2t = prep_inputs(x, weight)
    return [
        {"xs128": xs128,
         "xs66": xs66,
         "wst1": np.ascontiguousarray(wst1[:, i * ks:(i + 1) * ks]),
         "wst2": np.ascontiguousarray(wst2[:, i * ks:(i + 1) * ks]),
         "x2": x2t}
        for i in range(NCORES)
    ]


def kernel(x, weight):
    nc = _get_nc()
    in_maps = make_in_maps(x, weight)
    res = run_bass_kernel_spmd(nc, in_maps, core_ids=list(range(NCORES)))
    return np.concatenate([res.results[i]["out"] for i in range(NCORES)], axis=1)



# revision 2
# speedup vs baseline: 308144.9517x; 308144.9517x over previous
"""Euclidean distance (cdist) kernel for Trainium2, 8 NeuronCores.

out[b, j] = || x[b, :] - weight[:, j] ||_2   for x [4096, 64], weight [64, 50000].

Sharding (per hint): K = 50000 split into 8 slabs of 6250, one per core
(tensor-parallel over prototypes); x replicated; no cross-core reduction.

Math: dist^2 = x2[b] + w2[j] - 2*x@w, fused into ONE fp32r matmul by
augmenting the contraction dim (D=64 of 128 partitions free):

  lhsT = [-2x^T; 1]  [65, B]     rhs = [w; w2]  [65, KS]
  PSUM = -2 x@w + w2            (PE rounds operands to 11-bit mantissa;
                                 rel err ~1.6e-4, tolerance is 2e-2)
  ScalarE: out = sqrt(PSUM + x2[b])  (x2 as exact per-partition bias).

The kernel is HBM-store-bound: 102.4 MB of fp32 output per core vs
~2.7 MB of inputs. Loads issue on the ACT HWDGE ring (nc.scalar),
stores on the SP ring (nc.sync), so the initial weight load never
stalls the store stream. Per core: 32 b-tiles of 128 rows; per b-tile
13 matmuls of <=512 cols into 2048-col PSUM tiles; one contiguous
3.2 MB DMA store per b-tile (chunked on the first tile to start the
store pipeline early).
"""

import numpy as np
from contextlib import ExitStack

import concourse.bass as bass
import concourse.bacc as bacc
import concourse.tile as tile
from concourse import mybir
from concourse.bass_utils import run_bass_kernel_spmd

B, D, K = 4096, 64, 50000
NCORES = 8
KS = K // NCORES  # 6250 columns per core
P = 128
JT = 512          # matmul free-dim tile (one PSUM bank of fp32)
DL = D + 1        # 65: contraction rows ([-2x; 1] vs [w; w2])

F32 = mybir.dt.float32
F32R = mybir.dt.float32r


def build_nc(b=B, ks=KS):
    nbt = b // P
    nc = bacc.Bacc("TRN2", target_bir_lowering=False, debug=False)
    xst = nc.dram_tensor("xst", [DL, b], F32R, kind="ExternalInput").ap()
    wst = nc.dram_tensor("wst", [DL, ks], F32R, kind="ExternalInput").ap()
    x2 = nc.dram_tensor("x2", [P, nbt], F32, kind="ExternalInput").ap()
    out = nc.dram_tensor("out", [b, ks], F32, kind="ExternalOutput").ap()

    CHUNK = 4 * JT  # 2048: one 4-bank PSUM tile, one ACT instruction
    chunks = [(c0, min(CHUNK, ks - c0)) for c0 in range(0, ks, CHUNK)]

    with tile.TileContext(nc) as tc:
        with ExitStack() as ctx:
            singles = ctx.enter_context(tc.tile_pool(name="singles", bufs=1))
            outp = ctx.enter_context(tc.tile_pool(name="outp", bufs=3))
            psum = ctx.enter_context(tc.tile_pool(name="psum", bufs=2, space="PSUM"))

            wst_sb = singles.tile([DL, ks], F32R)
            xst_sb = singles.tile([DL, b], F32R)
            x2_sb = singles.tile([P, nbt], F32)

            # Loads on the ACT HWDGE ring, criticality order: the first
            # j-tile's weights + b-tile-0 x gate the first matmul.
            nc.scalar.dma_start(out=wst_sb[:, 0:JT], in_=wst[:, 0:JT])
            nc.scalar.dma_start(out=xst_sb[:, 0:P], in_=xst[:, 0:P])
            nc.scalar.dma_start(out=x2_sb, in_=x2)
            nc.scalar.dma_start(out=wst_sb[:, JT:CHUNK], in_=wst[:, JT:CHUNK])
            nc.scalar.dma_start(out=xst_sb[:, P:b], in_=xst[:, P:b])
            nc.scalar.dma_start(out=wst_sb[:, CHUNK:ks], in_=wst[:, CHUNK:ks])

            for ib in range(nbt):
                # Chunked store only on the first b-tile (starts the store
                # pipeline early); whole-row 3.2 MB stores otherwise.
                chunked_store = ib == 0
                ot = outp.tile([P, ks], F32)
                for ic, (c0, cn) in enumerate(chunks):
                    pt = psum.tile([P, CHUNK], F32)
                    for jj in range(0, cn, JT):
                        jn = min(JT, cn - jj)
                        nc.tensor.matmul(
                            pt[:, jj:jj + jn],
                            xst_sb[:, ib * P:(ib + 1) * P],
                            wst_sb[:, c0 + jj:c0 + jj + jn],
                            start=True,
                            stop=True,
                        )
                    nc.scalar.activation(
                        ot[:, c0:c0 + cn],
                        pt[:, :cn],
                        mybir.ActivationFunctionType.Sqrt,
                        bias=x2_sb[:, ib:ib + 1],
                        scale=1.0,
                    )
                    if chunked_store:
                        nc.sync.dma_start(
                            out=out[ib * P:(ib + 1) * P, c0:c0 + cn],
                            in_=ot[:, c0:c0 + cn],
                        )
                if not chunked_store:
                    nc.sync.dma_start(out=out[ib * P:(ib + 1) * P, :], in_=ot)
    nc.compile()
    return nc


def prep_inputs(x, weight):
    """Host-side prep: augmented-contraction operand matrices."""
    x = np.ascontiguousarray(x, dtype=np.float32)
    weight = np.ascontiguousarray(weight, dtype=np.float32)
    b, d = x.shape
    k = weight.shape[1]
    x2 = (x.astype(np.float64) ** 2).sum(axis=1).astype(np.float32)
    w2 = (weight.astype(np.float64) ** 2).sum(axis=0).astype(np.float32)

    xst = np.empty((DL, b), dtype=np.float32)
    xst[:d] = (-2.0 * x).T
    xst[d] = 1.0
    wst = np.empty((DL, k), dtype=np.float32)
    wst[:d] = weight
    wst[d] = w2
    x2t = np.ascontiguousarray(x2.reshape(b // P, P).T)  # [P, NBT]
    return xst, wst, x2t


_nc_cache = {}


def _get_nc():
    if "nc" not in _nc_cache:
        _nc_cache["nc"] = build_nc()
    return _nc_cache["nc"]


def make_in_maps(x, weight, ks=KS):
    xst, wst, x2t = prep_inputs(x, weight)
    return [
        {"xst": xst,
         "wst": np.ascontiguousarray(wst[:, i * ks:(i + 1) * ks]),
         "x2": x2t}
        for i in range(NCORES)
    ]


def kernel(x, weight):
    nc = _get_nc()
    in_maps = make_in_maps(x, weight)
    res = run_bass_kernel_spmd(nc, in_maps, core_ids=list(range(NCORES)))
    return np.concatenate([res.results[i]["out"] for i in range(NCORES)], axis=1)
